# revision 1
# baseline (speedup 1.0000x reference)
"""DCNv2 deformable-conv alignment kernel for 8 Trainium2 NeuronCores.

Sharding: core i handles (b = i//2, row-half = i%2) of the B=4, H=128 input.
Each core computes its half-image rows end-to-end:
  conv1 (128->64, 3x3) + lrelu -> conv2 (64->216, 3x3) -> offsets/mask
  -> bilinear sampling of nbr via GPSIMD ap_gather -> modulated DCN matmul
  -> bias + lrelu.
"""
import sys

for _p in ("/opt/trn_rl_repo", "/root/.axon_site/_ro/trn_rl_repo"):
    if _p not in sys.path:
        sys.path.insert(0, _p)

import numpy as np

NF, G, K = 64, 8, 3
KK = K * K
CG = NF // G
B, H, W = 4, 128, 128
N_CORES = 8
HALF = H // 2          # rows per core
CH = 512               # positions per chunk (4 image rows)
RPC = CH // W          # rows per chunk = 4
NCHUNK = HALF * W // CH  # 16
NE = H * W             # gather source elements per partition
NI = CH * 4            # gather indices per instruction (4 corners)

_compiled = None


def _build_program():
    import concourse.bacc as bacc
    import concourse.mybir as mybir
    import concourse.tile as tile
    from concourse.tile_rust import add_dep_helper

    dt = mybir.dt
    Alu = mybir.AluOpType
    Act = mybir.ActivationFunctionType

    nc = bacc.Bacc("TRN2", target_bir_lowering=False, debug=False,
                   num_devices=N_CORES)

    # ---- DRAM I/O ----
    conv_in_d = nc.dram_tensor("conv_in", [128, 68 * 130], dt.float32, kind="ExternalInput").ap()
    nbr_g_d = nc.dram_tensor("nbr_g", [128, NE], dt.float32, kind="ExternalInput").ap()
    w1_d = nc.dram_tensor("w1", [128, KK * 64], dt.float32, kind="ExternalInput").ap()
    w2_d = nc.dram_tensor("w2", [64, 3 * KK * 72], dt.float32, kind="ExternalInput").ap()
    w3_d = nc.dram_tensor("w3", [128, KK * 64], dt.float32, kind="ExternalInput").ap()
    by_d = nc.dram_tensor("by", [72, 1], dt.float32, kind="ExternalInput").ap()
    bx_d = nc.dram_tensor("bx", [72, 1], dt.float32, kind="ExternalInput").ap()
    bm_d = nc.dram_tensor("bm", [72, 1], dt.float32, kind="ExternalInput").ap()
    b1_d = nc.dram_tensor("b1", [64, 1], dt.float32, kind="ExternalInput").ap()
    b3_d = nc.dram_tensor("b3", [64, 1], dt.float32, kind="ExternalInput").ap()
    e0_d = nc.dram_tensor("e0", [64, 1], dt.float32, kind="ExternalInput").ap()
    e65_d = nc.dram_tensor("e65", [64, 1], dt.float32, kind="ExternalInput").ap()
    wrep_d = nc.dram_tensor("wrep", [72, KK * 128], dt.float32, kind="ExternalInput").ap()
    ramp_d = nc.dram_tensor("ramp", [128, CH], dt.float32, kind="ExternalInput").ap()
    hloc_d = nc.dram_tensor("hloc", [128, CH], dt.float32, kind="ExternalInput").ap()
    out_d = nc.dram_tensor("out", [64, HALF * W], dt.float32, kind="ExternalOutput").ap()

    f32 = dt.float32

    def lrelu_stt(out_ap, in_ap):
        # out = max(0.1*x, x)
        nc.vector.scalar_tensor_tensor(out_ap, in_ap, 0.1, in_ap, Alu.mult, Alu.max)

    # Static SBUF for DMA-written / gather tiles: Tile pool slot reuse +
    # dynamic-HWDGE-queue writes are under-synchronized (race detector),
    # so give these fixed, never-reused addresses.
    idxw = nc.alloc_sbuf_tensor("idxw_s", [128, NI // 16], dt.int16).ap()
    g_out = nc.alloc_sbuf_tensor("g_out_s", [128, CH * 4], dt.float32).ap()

    with tile.TileContext(nc) as tc:
        with tc.tile_pool(name="const", bufs=1) as cpool, \
             tc.tile_pool(name="psum", bufs=1, space="PSUM") as ppool:

            # ---- persistent loads ----
            nbr_sb = cpool.tile([128, NE], f32)
            nc.sync.dma_start(nbr_sb[:], nbr_g_d[:])
            w1_sb = cpool.tile([128, KK * 64], f32)
            nc.sync.dma_start(w1_sb[:], w1_d[:])
            w2_sb = cpool.tile([64, 3 * KK * 72], f32)
            nc.sync.dma_start(w2_sb[:], w2_d[:])
            w3_sb = cpool.tile([128, KK * 64], f32)
            nc.sync.dma_start(w3_sb[:], w3_d[:])
            by_sb = cpool.tile([72, 1], f32)
            nc.sync.dma_start(by_sb[:], by_d[:])
            bx_sb = cpool.tile([72, 1], f32)
            nc.sync.dma_start(bx_sb[:], bx_d[:])
            bm_sb = cpool.tile([72, 1], f32)
            nc.sync.dma_start(bm_sb[:], bm_d[:])
            b1_sb = cpool.tile([64, 1], f32)
            nc.sync.dma_start(b1_sb[:], b1_d[:])
            b3_sb = cpool.tile([64, 1], f32)
            nc.sync.dma_start(b3_sb[:], b3_d[:])
            e0_sb = cpool.tile([64, 1], f32)
            nc.sync.dma_start(e0_sb[:], e0_d[:])
            e65_sb = cpool.tile([64, 1], f32)
            nc.sync.dma_start(e65_sb[:], e65_d[:])
            wrep_sb = cpool.tile([72, KK * 128], f32)
            nc.sync.dma_start(wrep_sb[:], wrep_d[:])
            ramp_sb = cpool.tile([128, CH], f32)
            nc.sync.dma_start(ramp_sb[:], ramp_d[:])
            hloc_sb = cpool.tile([128, CH], f32)
            nc.sync.dma_start(hloc_sb[:], hloc_d[:])
            # wloc = ramp - 128*hloc  (column index 0..127)
            wloc_sb = cpool.tile([128, CH], f32)
            nc.vector.scalar_tensor_tensor(wloc_sb[:], hloc_sb[:], -128.0,
                                           ramp_sb[:], Alu.mult, Alu.add)

            # ---- conv1: off_feat rows [-1, HALF+1) padded cols (130 wide) ----
            off_sb = cpool.tile([64, 66 * 130], f32)
            nc.vector.memset(off_sb[:], 0.0)
            off_v = off_sb[:].rearrange("p (r c) -> p r c", c=130)
            with tc.tile_pool(name="cin", bufs=1) as cinpool:
                conv_in_sb = cinpool.tile([128, 68 * 130], f32)
                nc.sync.dma_start(conv_in_sb[:], conv_in_d[:])
                cin_v = conv_in_sb[:].rearrange("p (r c) -> p r c", c=130)
                j0 = 0
                while j0 < 66:
                    nrow = min(4, 66 - j0)
                    ps1 = ppool.tile([64, nrow, 128], f32, tag="ps1")
                    for kt in range(KK):
                        ky, kx = kt // 3, kt % 3
                        rhs = cin_v[:, j0 + ky: j0 + ky + nrow, kx: kx + 128]
                        nc.tensor.matmul(ps1[:], w1_sb[:, kt * 64:(kt + 1) * 64],
                                         rhs, start=(kt == 0), stop=(kt == KK - 1))
                    scf = cinpool.tile([64, nrow, 128], f32, tag="scf")
                    nc.vector.tensor_scalar(scf[:], ps1[:], b1_sb[:, 0:1], None, Alu.add)
                    lrelu_stt(off_v[:, j0: j0 + nrow, 1:129], scf[:])
                    j0 += nrow
            # off_feat rows outside the image must be ZERO for conv2's
            # zero-padding semantics (row j=0 is global s-1; j=65 is s+65).
            nc.vector.tensor_scalar(off_sb[:, 0:130], off_sb[:, 0:130],
                                    e0_sb[:, 0:1], None, Alu.mult)
            nc.vector.tensor_scalar(off_sb[:, 65 * 130:66 * 130],
                                    off_sb[:, 65 * 130:66 * 130],
                                    e65_sb[:, 0:1], None, Alu.mult)

            # ---- per-chunk pipeline ----
            prev_gather = [None]
            with tc.tile_pool(name="work", bufs=1) as wpool:
                for c in range(NCHUNK):
                    # conv2 -> three field psums [72, CH]
                    ps_f = []
                    for f in range(3):
                        psf = ppool.tile([72, RPC, 128], f32, tag=f"ps2_{f}")
                        for kt in range(KK):
                            ky, kx = kt // 3, kt % 3
                            rhs = off_v[:, c * RPC + ky: c * RPC + ky + RPC, kx: kx + 128]
                            nc.tensor.matmul(
                                psf[:],
                                w2_sb[:, (f * KK + kt) * 72:(f * KK + kt + 1) * 72],
                                rhs, start=(kt == 0), stop=(kt == KK - 1))
                        ps_f.append(psf)

                    qy = wpool.tile([72, CH], f32, tag="qy")
                    nc.vector.tensor_scalar(qy[:], ps_f[0][:].rearrange("p a b -> p (a b)"),
                                            by_sb[:, 0:1], None, Alu.add)
                    qx = wpool.tile([72, CH], f32, tag="qx")
                    nc.vector.tensor_scalar(qx[:], ps_f[1][:].rearrange("p a b -> p (a b)"),
                                            bx_sb[:, 0:1], None, Alu.add)
                    msk = wpool.tile([72, CH], f32, tag="msk")
                    nc.scalar.activation(msk[:], ps_f[2][:].rearrange("p a b -> p (a b)"),
                                         Act.Sigmoid, bias=bm_sb[:, 0:1], scale=1.0)

                    # floor(qy) -> fy ; wy = qy - fy   (exact for any converter rounding)
                    def floor_of(q, tag):
                        ti = wpool.tile([72, CH], dt.int32, tag="fl_i32")
                        nc.vector.tensor_copy(ti[:], q[:])
                        tf = wpool.tile([72, CH], f32, tag="fl_f32")
                        nc.vector.tensor_copy(tf[:], ti[:])
                        gg = wpool.tile([72, CH], f32, tag="fl_gt")
                        nc.vector.tensor_tensor(gg[:], tf[:], q[:], Alu.is_gt)
                        fl = wpool.tile([72, CH], f32, tag=tag)
                        nc.vector.tensor_tensor(fl[:], tf[:], gg[:], Alu.subtract)
                        return fl

                    fy = floor_of(qy, "fy")
                    fx = floor_of(qx, "fx")
                    wy = wpool.tile([72, CH], f32, tag="wy")
                    nc.vector.tensor_tensor(wy[:], qy[:], fy[:], Alu.subtract)
                    wx = wpool.tile([72, CH], f32, tag="wx")
                    nc.vector.tensor_tensor(wx[:], qx[:], fx[:], Alu.subtract)

                    # validity: hloc/ramp are chunk-0 GLOBAL values (s baked in by
                    # host); chunk c shifts rows by c*RPC, folded into the scalar
                    # bounds and corner offsets below.
                    R0 = c * RPC
                    t2y = wpool.tile([72, CH], f32, tag="t2y")
                    nc.vector.tensor_tensor(t2y[:], hloc_sb[:72, :], fy[:], Alu.add)
                    t2x = wpool.tile([72, CH], f32, tag="t2x")
                    nc.vector.tensor_tensor(t2x[:], wloc_sb[:72, :], fx[:], Alu.add)

                    def valid(t2, lo, hi, tag):
                        cc = wpool.tile([72, CH], f32, tag="v_clip")
                        nc.vector.tensor_scalar(cc[:], t2[:], float(hi), float(lo),
                                                Alu.min, Alu.max)
                        vv = wpool.tile([72, CH], f32, tag=tag)
                        nc.vector.tensor_tensor(vv[:], cc[:], t2[:], Alu.is_equal)
                        return vv

                    vy0 = valid(t2y, 0 - R0, 127 - R0, "vy0")
                    vy1 = valid(t2y, -1 - R0, 126 - R0, "vy1")
                    vx0 = valid(t2x, 0, 127, "vx0")
                    vx1 = valid(t2x, -1, 126, "vx1")

                    # corner weights (validity and mask folded in)
                    uy0 = wpool.tile([72, CH], f32, tag="uy0")
                    nc.vector.tensor_scalar(uy0[:], wy[:], -1.0, 1.0, Alu.mult, Alu.add)
                    nc.vector.tensor_tensor(uy0[:], uy0[:], vy0[:], Alu.mult)
                    nc.vector.tensor_tensor(uy0[:], uy0[:], msk[:], Alu.mult)
                    uy1 = wpool.tile([72, CH], f32, tag="uy1")
                    nc.vector.tensor_tensor(uy1[:], wy[:], vy1[:], Alu.mult)
                    nc.vector.tensor_tensor(uy1[:], uy1[:], msk[:], Alu.mult)
                    ux0 = wpool.tile([72, CH], f32, tag="ux0")
                    nc.vector.tensor_scalar(ux0[:], wx[:], -1.0, 1.0, Alu.mult, Alu.add)
                    nc.vector.tensor_tensor(ux0[:], ux0[:], vx0[:], Alu.mult)
                    ux1 = wpool.tile([72, CH], f32, tag="ux1")
                    nc.vector.tensor_tensor(ux1[:], wx[:], vx1[:], Alu.mult)

                    # cu interleaved [72, CH, 4]
                    cu = wpool.tile([72, CH, 4], f32, tag="cu")
                    nc.vector.tensor_tensor(cu[:, :, 0], uy0[:], ux0[:], Alu.mult)
                    nc.vector.tensor_tensor(cu[:, :, 1], uy0[:], ux1[:], Alu.mult)
                    nc.vector.tensor_tensor(cu[:, :, 2], uy1[:], ux0[:], Alu.mult)
                    nc.vector.tensor_tensor(cu[:, :, 3], uy1[:], ux1[:], Alu.mult)

                    # flat gather indices.  true flat = (h_g + fy + dy')*128 +
                    # (w + fx + dx') = ramp_local + 128*(hbase part in hloc) ...
                    # since hloc is global h already: flat = (hloc+fy+dy')*128 +
                    # (wloc+fx+dx')  = [hloc*128 + wloc] + 128*fy + fx + off
                    # host supplies ramp = hloc*128 + wloc (global flat idx).
                    base = wpool.tile([72, CH], f32, tag="base")
                    nc.vector.tensor_scalar(base[:], fy[:], 128.0, None, Alu.mult)
                    nc.vector.tensor_tensor(base[:], base[:], fx[:], Alu.add)
                    nc.vector.tensor_tensor(base[:], base[:], ramp_sb[:72, :], Alu.add)

                    idx16 = []
                    for cidx, off in enumerate((0.0, 1.0, 128.0, 129.0)):
                        icf = wpool.tile([72, CH], f32, tag="idx_f")
                        nc.vector.tensor_scalar(icf[:], base[:], off + c * CH,
                                                float(NE - 1), Alu.add, Alu.min)
                        nc.vector.tensor_scalar(icf[:], icf[:], -16384.0, None, Alu.max)
                        ici = wpool.tile([72, CH], dt.int32, tag="idx_i32")
                        nc.vector.tensor_copy(ici[:], icf[:])
                        i16 = wpool.tile([72, CH], dt.int16, tag=f"idx16_{cidx}")
                        nc.vector.tensor_copy(i16[:], ici[:])
                        idx16.append(i16)

                    # per-tap: build wrapped idx, gather, weight, reduce, matmul
                    dcn_ps = ppool.tile([64, CH], f32, tag="dcn_ps")
                    for kt in range(KK):
                        # wrapped idx layout: list element j = pos*4 + corner
                        # lives at partition (j%16), column j//16; partition
                        # p = 4*pf + cidx holds corner cidx of positions
                        # {t*4 + pf}, i.e. a stride-4 slice of the idx field.
                        # Tile's dep tracking under-covers strided-partition
                        # DMA writes, so wire explicit deps to the gather.
                        idx_dmas = []
                        for cidx in range(4):
                            srcv = idx16[cidx][kt * 8:(kt + 1) * 8, :].rearrange(
                                "p (s four) -> p four s", four=4)
                            for pf in range(4):
                                d = nc.scalar.dma_start(idxw[4 * pf + cidx::16, :],
                                                        srcv[:, pf, :])
                                if prev_gather[0] is not None:
                                    add_dep_helper(d.ins, prev_gather[0].ins, True,
                                                   "idxw WAR vs prev gather")
                                idx_dmas.append(d)

                        gth = nc.gpsimd.ap_gather(out_ap=g_out[:], in_ap=nbr_sb[:],
                                                  idxs_ap=idxw[:], channels=128,
                                                  num_elems=NE, d=1, num_idxs=NI)
                        for d in idx_dmas:
                            add_dep_helper(gth.ins, d.ins, True, "gather RAW on idxw")
                        prev_gather[0] = gth
                        # replicate cu rows to the 16-partition gather layout
                        # via one-hot matmul (avoids the DMA-queue storm)
                        cuf = cu[:, :, :].rearrange("p a b -> p (a b)")
                        for t in range(4):
                            rp = ppool.tile([128, 512], f32, tag="rep_ps")
                            nc.tensor.matmul(rp[:],
                                             wrep_sb[:, kt * 128:(kt + 1) * 128],
                                             cuf[:, t * 512:(t + 1) * 512],
                                             start=True, stop=True)
                            nc.vector.tensor_tensor(
                                g_out[:, t * 512:(t + 1) * 512],
                                g_out[:, t * 512:(t + 1) * 512], rp[:], Alu.mult)
                        samp = wpool.tile([128, CH], f32, tag="samp")
                        nc.vector.tensor_reduce(
                            samp[:], g_out[:].rearrange("p (pos four) -> p pos four", four=4),
                            axis=mybir.AxisListType.X, op=Alu.add)
                        nc.tensor.matmul(dcn_ps[:], w3_sb[:, kt * 64:(kt + 1) * 64],
                                         samp[:], start=(kt == 0), stop=(kt == KK - 1))

                    oc = wpool.tile([64, CH], f32, tag="oc")
                    nc.vector.tensor_scalar(oc[:], dcn_ps[:], b3_sb[:, 0:1], None, Alu.add)
                    ob = wpool.tile([64, CH], f32, tag="ob")
                    lrelu_stt(ob[:], oc[:])
                    nc.sync.dma_start(out_d[:, c * CH:(c + 1) * CH], ob[:])

    nc.compile()
    return nc


def _prep_inputs(nbr, ref, w_off1, b_off1, w_om, b_om, w_dcn, b_dcn):
    """Build the 8 per-core input dicts."""
    in_maps = []
    # weights shared by all cores
    w1 = np.zeros((128, KK * 64), np.float32)
    for kt in range(KK):
        ky, kx = kt // 3, kt % 3
        w1[:, kt * 64:(kt + 1) * 64] = w_off1[:, :, ky, kx].T  # [128in, 64out]
    w2 = np.zeros((64, 3 * KK * 72), np.float32)
    for f in range(3):
        for kt in range(KK):
            ky, kx = kt // 3, kt % 3
            # m-dim p = k*8+g  ->  om channel f*72 + g*9 + k
            blk = np.zeros((64, 72), np.float32)
            for k in range(KK):
                for g in range(G):
                    blk[:, k * 8 + g] = w_om[f * 72 + g * KK + k, :, ky, kx]
            w2[:, (f * KK + kt) * 72:(f * KK + kt + 1) * 72] = blk
    w3 = np.zeros((128, KK * 64), np.float32)
    wd = w_dcn.reshape(64, G, CG, 3, 3)
    for kt in range(KK):
        ky, kx = kt // 3, kt % 3
        blk = np.zeros((128, 64), np.float32)
        for g in range(G):
            for j in range(CG):
                blk[16 * g + j, :] = wd[:, g, j, ky, kx]
        w3[:, kt * 64:(kt + 1) * 64] = blk

    wrep = np.zeros((72, KK * 128), np.float32)
    for kt in range(KK):
        for m in range(128):
            wrep[kt * 8 + m // 16, kt * 128 + m] = 1.0

    dy = np.repeat(np.arange(3) - 1, 3).astype(np.float32)  # per tap k
    dx = np.tile(np.arange(3) - 1, 3).astype(np.float32)
    by = np.zeros((72, 1), np.float32)
    bx = np.zeros((72, 1), np.float32)
    bm = np.zeros((72, 1), np.float32)
    for k in range(KK):
        for g in range(G):
            p = k * 8 + g
            by[p, 0] = b_om[0 * 72 + g * KK + k] + dy[k]
            bx[p, 0] = b_om[1 * 72 + g * KK + k] + dx[k]
            bm[p, 0] = b_om[2 * 72 + g * KK + k]
    b1 = b_off1.reshape(64, 1).astype(np.float32)
    b3 = b_dcn.reshape(64, 1).astype(np.float32)

    for core in range(N_CORES):
        b = core // 2
        s = (core % 2) * HALF
        # conv input: concat channels, rows [s-2, s+66), zero pad, 130 cols
        ci = np.zeros((128, 68, 130), np.float32)
        cat = np.concatenate([nbr[b], ref[b]], axis=0)  # [128, H, W]
        r_lo, r_hi = s - 2, s + 66
        src_lo, src_hi = max(r_lo, 0), min(r_hi, H)
        ci[:, src_lo - r_lo: src_hi - r_lo, 1:129] = cat[:, src_lo:src_hi, :]
        # gather source layout
        ng = np.zeros((128, NE), np.float32)
        for g in range(G):
            for j in range(16):
                ng[16 * g + j] = nbr[b, CG * g + (j % CG)].reshape(-1)
        # chunk-0 global ramps: hloc = global row of position (s baked in);
        # ramp = global flat index.  Chunk c's shift (c*RPC rows = c*CH flat)
        # is folded into scalar constants inside the program.
        pos = np.arange(CH, dtype=np.float32)
        hl = s + pos // W
        fl = hl * W + (pos % W)
        e0 = np.full((64, 1), 0.0 if s == 0 else 1.0, np.float32)
        e65 = np.full((64, 1), 0.0 if s + HALF == H else 1.0, np.float32)
        in_maps.append(dict(
            conv_in=ci.reshape(128, -1), nbr_g=ng, w1=w1, w2=w2, w3=w3,
            by=by, bx=bx, bm=bm, b1=b1, b3=b3, e0=e0, e65=e65, wrep=wrep,
            hloc=np.broadcast_to(hl, (128, CH)).astype(np.float32).copy(),
            ramp=np.broadcast_to(fl, (128, CH)).astype(np.float32).copy(),
        ))
    return in_maps


def kernel(**inputs):
    global _compiled
    from concourse.bass_utils import run_bass_kernel_spmd

    if _compiled is None:
        _compiled = _build_program()
    nc = _compiled

    in_maps = _prep_inputs(
        inputs["nbr_fea_l"], inputs["ref_fea_l"], inputs["w_off1"],
        inputs["b_off1"], inputs["w_om"], inputs["b_om"],
        inputs["w_dcn"], inputs["b_dcn"])

    res = run_bass_kernel_spmd(nc, in_maps, core_ids=list(range(N_CORES)))
    out = np.zeros((B, NF, H, W), np.float32)
    for core in range(N_CORES):
        b = core // 2
        s = (core % 2) * HALF
        out[b, :, s:s + HALF, :] = res.results[core]["out"].reshape(64, HALF, W)
    return out


if __name__ == "__main__":
    rng = np.random.default_rng(0)
    print("smoke build only")
    _build_program()
    print("build ok")



# revision 2
# speedup vs baseline: 1.3017x; 1.3017x over previous
"""DCNv2 deformable-conv alignment kernel for 8 Trainium2 NeuronCores (v2).

Sharding: core i handles (b = i//2, row-half = i%2) of the B=4, H=128 input.

v2 redesign vs baseline:
- all matmuls in bf16 (4x PE throughput vs fp32)
- guard-band padded gather source (no validity masks; OOB corners read zeros)
- pair-gather: source is row-interleaved [A|B] pair copies in bf16, gather
  d=2 fetches an x-adjacent pair per index -> half the indices, half bytes
- per-chunk banded gather source view (32 rows) to cut num_elems
- index wrap for the gather is ONE contiguous SBUF->SBUF DMA per tap
  (c-major index layout chosen so the wrap is 128B-contiguous runs)
- corner weights replicated 8->128 partitions by a broadcast DMA (not matmul)
- corner reduction folded into the DCN matmul (4 accumulating matmuls/tap)
- positive-shifted coordinates so mod(x,1) == frac(x) on DVE
"""
import sys

for _p in ("/opt/trn_rl_repo", "/root/.axon_site/_ro/trn_rl_repo"):
    if _p not in sys.path:
        sys.path.insert(0, _p)

import numpy as np
import ml_dtypes

NF, G, K = 64, 8, 3
KK = K * K
CG = NF // G
B, H, W = 4, 128, 128
N_CORES = 8
HALF = H // 2

GP = 14                 # guard pad (rows/cols) around each core's band
WP = W + 2 * GP         # 156 padded width
HP = HALF + 2 * GP      # 92 padded rows
NPR = WP                # pairs per padded row (78 A-pairs + 78 B-pairs)
NE_AB = HP * NPR        # 14352 total pair elements
ROWS_BAND = 32
BAND = ROWS_BAND * NPR  # 4992 pair elems per gather view

CH = 512                # positions per chunk (4 rows)
RPC = CH // W           # 4
NCHUNK = HALF * W // CH # 16
NIDX = 2 * CH           # 1024 gather indices per tap (2 y-corners x 512 pos)

IDX_ON_ACT = True       # emit idx int16 conversion on scalar engine
FLOOR_TRUNC = False     # assume fp32->int32 cast truncates (test on HW)
CAST_ON_ACT = False     # run fp32<->int32 casts on the scalar engine

_compiled = None


def _build_program():
    import concourse.bacc as bacc
    import concourse.mybir as mybir
    import concourse.tile as tile
    from concourse.tile_rust import add_dep_helper
    from concourse.bass_types import AP

    def mk_ap(base_ap, dims):
        """Raw AP on base_ap's tensor+offset with explicit [stride, count]
        dims in flat element units (partition stride = row width)."""
        return AP(base_ap.tensor, base_ap.offset, dims)

    dt = mybir.dt
    Alu = mybir.AluOpType
    Act = mybir.ActivationFunctionType
    f32 = dt.float32
    f32r = dt.float32r
    bf16 = dt.bfloat16

    nc = bacc.Bacc("TRN2", target_bir_lowering=False, debug=False,
                   num_devices=N_CORES)

    # ---- DRAM I/O ----
    conv_in_d = nc.dram_tensor("conv_in", [128, 68 * 130], bf16, kind="ExternalInput").ap()
    nbrab_d = nc.dram_tensor("nbrab", [128, NE_AB * 2], bf16, kind="ExternalInput").ap()
    w1_d = nc.dram_tensor("w1", [128, KK * 64], bf16, kind="ExternalInput").ap()
    w2_d = nc.dram_tensor("w2", [64, 3 * KK * 72], bf16, kind="ExternalInput").ap()
    w3_d = nc.dram_tensor("w3", [128, KK * 64], bf16, kind="ExternalInput").ap()
    wrep_d = nc.dram_tensor("wrep", [72, KK * 128], f32r, kind="ExternalInput").ap()
    by_d = nc.dram_tensor("by", [72, 1], f32, kind="ExternalInput").ap()
    bx_d = nc.dram_tensor("bx", [72, 1], f32, kind="ExternalInput").ap()
    bm_d = nc.dram_tensor("bm", [72, 1], f32, kind="ExternalInput").ap()
    b1_d = nc.dram_tensor("b1", [64, 1], f32, kind="ExternalInput").ap()
    b3_d = nc.dram_tensor("b3", [64, 1], f32, kind="ExternalInput").ap()
    e0_d = nc.dram_tensor("e0", [64, 1], f32, kind="ExternalInput").ap()
    e65_d = nc.dram_tensor("e65", [64, 1], f32, kind="ExternalInput").ap()
    rampc_d = nc.dram_tensor("rampc", [128, CH], f32, kind="ExternalInput").ap()
    c156_d = nc.dram_tensor("c156", [72, 1], f32, kind="ExternalInput").ap()
    out_d = nc.dram_tensor("out", [64, HALF * W], f32, kind="ExternalOutput").ap()

    def lrelu_stt(out_ap, in_ap):
        nc.vector.scalar_tensor_tensor(out_ap, in_ap, 0.1, in_ap, Alu.mult, Alu.max)

    # Static SBUF for gather-adjacent tensors (partition-strided DMA writes
    # are under-tracked by Tile): fixed addresses + explicit deps.
    # Taps are processed in groups of TG=3: one gather per group; corner
    # weights are replicated 8->128 partitions by a one-hot fp32r matmul.
    TG = 3
    NI3 = TG * NIDX          # 3072 indices per gather
    idxw_s = [nc.alloc_sbuf_tensor(f"idxw{i}", [128, NI3 // 16], dt.int16).ap()
              for i in range(2)]
    gout_s = [nc.alloc_sbuf_tensor(f"gout{i}", [128, NI3 * 2], bf16).ap()
              for i in range(2)]

    with tile.TileContext(nc) as tc:
        with tc.tile_pool(name="const", bufs=1) as cpool, \
             tc.tile_pool(name="psum", bufs=2, space="PSUM") as ppool, \
             tc.tile_pool(name="psum_rep", bufs=1, space="PSUM") as ppool_r:

            # ---- persistent loads ----
            nbrab_sb = cpool.tile([128, NE_AB * 2], bf16)
            nc.sync.dma_start(nbrab_sb[:], nbrab_d[:])
            w1_sb = cpool.tile([128, KK * 64], bf16)
            nc.sync.dma_start(w1_sb[:], w1_d[:])
            w2_sb = cpool.tile([64, 3 * KK * 72], bf16)
            nc.sync.dma_start(w2_sb[:], w2_d[:])
            w3_sb = cpool.tile([128, KK * 64], bf16)
            nc.sync.dma_start(w3_sb[:], w3_d[:])
            wrep_sb = cpool.tile([72, KK * 128], f32r)
            nc.sync.dma_start(wrep_sb[:], wrep_d[:])
            by_sb = cpool.tile([72, 1], f32)
            nc.sync.dma_start(by_sb[:], by_d[:])
            bx_sb = cpool.tile([72, 1], f32)
            nc.sync.dma_start(bx_sb[:], bx_d[:])
            bm_sb = cpool.tile([72, 1], f32)
            nc.sync.dma_start(bm_sb[:], bm_d[:])
            b1_sb = cpool.tile([64, 1], f32)
            nc.sync.dma_start(b1_sb[:], b1_d[:])
            b3_sb = cpool.tile([64, 1], f32)
            nc.sync.dma_start(b3_sb[:], b3_d[:])
            e0_sb = cpool.tile([64, 1], f32)
            nc.sync.dma_start(e0_sb[:], e0_d[:])
            e65_sb = cpool.tile([64, 1], f32)
            nc.sync.dma_start(e65_sb[:], e65_d[:])
            rampc_sb = cpool.tile([128, CH], f32)
            nc.sync.dma_start(rampc_sb[:], rampc_d[:])
            c156_sb = cpool.tile([72, 1], f32)
            nc.sync.dma_start(c156_sb[:], c156_d[:])

            # ---- conv1 -> off_feat rows [-1, HALF+1), 130-wide padded, bf16 ----
            off_sb = cpool.tile([64, 66 * 130], bf16)
            nc.vector.memset(off_sb[:], 0.0)
            off_v = off_sb[:].rearrange("p (r c) -> p r c", c=130)
            with tc.tile_pool(name="cin", bufs=1) as cinpool:
                conv_in_sb = cinpool.tile([128, 68 * 130], bf16)
                nc.sync.dma_start(conv_in_sb[:], conv_in_d[:])
                cin_v = conv_in_sb[:].rearrange("p (r c) -> p r c", c=130)
                j0 = 0
                while j0 < 66:
                    nrow = min(4, 66 - j0)
                    ps1 = ppool.tile([64, nrow, 128], f32, tag="dcn_ps")
                    for kt in range(KK):
                        ky, kx = kt // 3, kt % 3
                        rhs = cin_v[:, j0 + ky: j0 + ky + nrow, kx: kx + 128]
                        nc.tensor.matmul(ps1[:], w1_sb[:, kt * 64:(kt + 1) * 64],
                                         rhs, start=(kt == 0), stop=(kt == KK - 1))
                    scf = cinpool.tile([64, nrow, 128], f32, tag="scf")
                    nc.vector.tensor_scalar(scf[:], ps1[:], b1_sb[:, 0:1], None, Alu.add)
                    lrelu_stt(off_v[:, j0: j0 + nrow, 1:129], scf[:])
                    j0 += nrow
            # zero off rows outside the image (conv2 zero-padding semantics)
            nc.vector.tensor_scalar(off_sb[:, 0:130], off_sb[:, 0:130],
                                    e0_sb[:, 0:1], None, Alu.mult)
            nc.vector.tensor_scalar(off_sb[:, 65 * 130:66 * 130],
                                    off_sb[:, 65 * 130:66 * 130],
                                    e65_sb[:, 0:1], None, Alu.mult)

            # ---- per-chunk pipeline ----
            gathers, mults, dcn_last, wraps, reps = [], [], [], [], []
            with tc.tile_pool(name="work", bufs=1) as wpool, \
                 tc.tile_pool(name="work2", bufs=2) as wpool2:
                for c in range(NCHUNK):
                    # conv2: one shared psum tag cycled across the 3 fields
                    outs_f = []
                    for f, (bias_ap, fun, tagn) in enumerate(
                            ((by_sb, Act.Identity, "qy"),
                             (bx_sb, Act.Identity, "qx"),
                             (bm_sb, Act.Sigmoid, "msk"))):
                        psf = ppool.tile([72, RPC, 128], f32, tag="ps2")
                        for kt in range(KK):
                            ky, kx = kt // 3, kt % 3
                            rhs = off_v[:, c * RPC + ky: c * RPC + ky + RPC, kx: kx + 128]
                            nc.tensor.matmul(
                                psf[:],
                                w2_sb[:, (f * KK + kt) * 72:(f * KK + kt + 1) * 72],
                                rhs, start=(kt == 0), stop=(kt == KK - 1))
                        ot = wpool.tile([72, CH], f32, tag=tagn)
                        nc.scalar.activation(ot[:], psf[:].rearrange("p a b -> p (a b)"),
                                             fun, bias=bias_ap[:, 0:1], scale=1.0)
                        outs_f.append(ot)
                    qy, qx, msk = outs_f

                    # frac/floor via int cast (coords strictly positive).
                    # FLOOR_TRUNC assumes the fp32->int32 cast truncates; the
                    # exact path corrects for round-to-nearest converters.
                    def cast_int_back(q, tag):
                        """round/trunc q -> int32 -> f32, on ACT or DVE."""
                        ti = wpool.tile([72, CH], dt.int32, tag="fl_i32")
                        tf = wpool.tile([72, CH], f32, tag=tag + "_f")
                        if CAST_ON_ACT:
                            nc.scalar.activation(ti[:], q[:], Act.Identity,
                                                 bias=0.0, scale=1.0)
                            nc.scalar.activation(tf[:], ti[:], Act.Identity,
                                                 bias=0.0, scale=1.0)
                        else:
                            nc.vector.tensor_copy(ti[:], q[:])
                            nc.vector.tensor_copy(tf[:], ti[:])
                        return tf

                    def floor_frac(q, tag):
                        tf = cast_int_back(q, tag)
                        if FLOOR_TRUNC:
                            fl = tf
                        else:
                            gg = wpool.tile([72, CH], f32, tag="fl_gt")
                            nc.vector.tensor_tensor(gg[:], tf[:], q[:], Alu.is_gt)
                            fl = wpool.tile([72, CH], f32, tag=tag)
                            nc.vector.tensor_tensor(fl[:], tf[:], gg[:], Alu.subtract)
                        w = wpool.tile([72, CH], f32, tag=tag + "_w")
                        nc.vector.tensor_tensor(w[:], q[:], fl[:], Alu.subtract)
                        return fl, w

                    fyf, wy = floor_frac(qy, "fy")
                    fxf, wx = floor_frac(qx, "fx")

                    # pair index: pidx = fyf*156 + (ramp + fxf*0.5) + 155*parh
                    # parh = frac(hx) in {0, 0.5}; |hx - round(hx)| is correct
                    # for either converter rounding mode.
                    hx = wpool.tile([72, CH], f32, tag="hx")
                    nc.vector.scalar_tensor_tensor(hx[:], fxf[:], 0.5,
                                                   rampc_sb[:72, :], Alu.mult, Alu.add)
                    pf_ = cast_int_back(hx, "pr")
                    pd = wpool.tile([72, CH], f32, tag="pr_d")
                    nc.vector.tensor_tensor(pd[:], hx[:], pf_[:], Alu.subtract)
                    # pd in {0, +-0.5}; pd^2 in {0, 0.25} -> parity term
                    sq = wpool.tile([72, CH], f32, tag="sq")
                    nc.vector.tensor_tensor(sq[:], pd[:], pd[:], Alu.mult)
                    hxp = wpool.tile([72, CH], f32, tag="hxp")
                    nc.vector.scalar_tensor_tensor(hxp[:], sq[:], float(2 * (NPR - 1)),
                                                   hx[:], Alu.mult, Alu.add)
                    pidx = wpool.tile([72, CH], f32, tag="pidx")
                    nc.vector.scalar_tensor_tensor(pidx[:], fyf[:], float(NPR),
                                                   hxp[:], Alu.mult, Alu.add)

                    # idx2: c-major int16 [72, 2*CH]; band start (4c rows) cancels
                    # the chunk shift, so bias is just the y-corner offset.
                    idx2 = wpool2.tile([72, 2 * CH], dt.int16, tag="idx2")
                    idx_ins = []
                    for yc in range(2):
                        if IDX_ON_ACT:
                            bias_v = 0.0 if yc == 0 else c156_sb[:, 0:1]
                            d = nc.scalar.activation(
                                idx2[:, yc * CH:(yc + 1) * CH], pidx[:],
                                Act.Identity, bias=bias_v, scale=1.0)
                        else:
                            d = nc.vector.tensor_scalar(
                                idx2[:, yc * CH:(yc + 1) * CH], pidx[:],
                                float(yc * NPR), None, Alu.add)
                        # idx2 slot reuse vs the (untracked) wrap-DMA reads of
                        # the chunk two back (KK wrap DMAs per chunk)
                        for w in wraps[(c - 2) * KK:(c - 1) * KK] if c >= 2 else []:
                            add_dep_helper(d.ins, w.ins, True, "idx2 WAR wrap")
                        idx_ins.append(d)

                    # corner weights -> cu_store bf16, layout col =
                    # xl*32 + yc*16 + r*4 + xh*2 + xc  (matches gather j-order)
                    t = wpool.tile([72, CH], f32, tag="t")
                    nc.vector.tensor_tensor(t[:], wy[:], msk[:], Alu.mult)
                    uy0 = wpool.tile([72, CH], f32, tag="uy0")
                    nc.vector.tensor_tensor(uy0[:], msk[:], t[:], Alu.subtract)
                    ux0 = wpool.tile([72, CH], f32, tag="ux0")
                    nc.scalar.activation(ux0[:], wx[:], Act.Identity, bias=1.0, scale=-1.0)

                    cu_store = wpool2.tile([72, CH * 4], f32r, tag="cu")
                    # view [p, yc, xc, r, xh, xl] of the j-ordered flat layout
                    cu_v = cu_store[:].rearrange(
                        "p (xl yc r xh xc) -> p xl yc r xh xc",
                        xl=64, yc=2, r=RPC, xh=2, xc=2).transpose(
                        [0, 2, 5, 3, 4, 1])
                    cu_ins = []
                    for (yc, xc, a_ap, b_ap) in (
                            (0, 0, uy0, ux0), (0, 1, uy0, wx),
                            (1, 0, t, ux0), (1, 1, t, wx)):
                        d = nc.vector.tensor_tensor(
                            cu_v[:, yc, xc],
                            a_ap[:].rearrange("p (r xh xl) -> p r xh xl",
                                              r=RPC, xh=2),
                            b_ap[:].rearrange("p (r xh xl) -> p r xh xl",
                                              r=RPC, xh=2),
                            Alu.mult)
                        cu_ins.append(d)

                    # band view of the gather source for this chunk
                    src_lo = c * RPC * NPR * 2
                    src_ap = nbrab_sb[:, src_lo: src_lo + BAND * 2]

                    dcn_ps = ppool.tile([64, CH], f32, tag="dcn_ps")
                    for grp in range(KK // TG):
                        gi = c * (KK // TG) + grp
                        bi = gi % 2
                        kt0 = grp * TG
                        grp_wraps = []
                        for t3 in range(TG):
                            kt = kt0 + t3
                            # wrap DMA: idxw3[16m+prow, t3*64+col] <-
                            #   idx2[kt*8+m, prow*64+col]   (3-dim manual APs)
                            d_b = idxw_s[bi][:, t3 * 64:(t3 + 1) * 64]
                            s_b = idx2[kt * 8:(kt + 1) * 8, :]
                            q = nc.sync if (len(wraps) % 2 == 0) else nc.scalar
                            wrap = q.dma_start(
                                mk_ap(d_b, [[NI3 // 16, 128], [1, 64]]),
                                mk_ap(s_b, [[NIDX, 8], [64, 16], [1, 64]]))
                            for d in idx_ins:
                                add_dep_helper(wrap.ins, d.ins, True, "wrap RAW idx2")
                            if len(gathers) >= 2:
                                add_dep_helper(wrap.ins, gathers[-2].ins, True,
                                               "idxw WAR prev gather")
                            wraps.append(wrap)
                            grp_wraps.append(wrap)

                        gth = nc.gpsimd.ap_gather(
                            out_ap=gout_s[bi][:], in_ap=src_ap,
                            idxs_ap=idxw_s[bi][:], channels=128,
                            num_elems=BAND, d=2, num_idxs=NI3)
                        for w in grp_wraps:
                            add_dep_helper(gth.ins, w.ins, True, "gather RAW idxw")
                        if len(mults) >= 4:
                            # last mult of the group two back read gout_s[bi];
                            # DVE is in-order so one dep covers all three
                            add_dep_helper(gth.ins, mults[-4].ins, True,
                                           "gout WAR prev mults")
                        gathers.append(gth)

                        for t3 in range(TG):
                            kt = kt0 + t3
                            # replicate cu rows (k,g) -> 16 partitions each via
                            # one-hot fp32r matmul (1 cyc/row on PE)
                            rep_ps = ppool_r.tile([128, CH * 4], f32, tag="rep_ps")
                            for h in range(4):
                                nc.tensor.matmul(
                                    rep_ps[:, h * CH:(h + 1) * CH],
                                    wrep_sb[:, kt * 128:(kt + 1) * 128],
                                    cu_store[:, h * CH:(h + 1) * CH],
                                    start=True, stop=True)
                            # weighting: gw = gout(tap slice) * rep  (bf16 out)
                            gw = wpool2.tile([128, CH * 4], bf16, tag="gw")
                            mult = nc.vector.tensor_tensor(
                                gw[:], gout_s[bi][:, t3 * CH * 4:(t3 + 1) * CH * 4],
                                rep_ps[:], Alu.mult)
                            add_dep_helper(mult.ins, gth.ins, True, "mult RAW gout")
                            mults.append(mult)

                            gw_v = gw[:].rearrange(
                                "p (xl yc r xh xc) -> p xl yc r xh xc",
                                xl=64, yc=2, r=RPC, xh=2, xc=2).transpose(
                                [0, 2, 5, 3, 4, 1])
                            for f in range(4):
                                nc.tensor.matmul(
                                    dcn_ps[:], w3_sb[:, kt * 64:(kt + 1) * 64],
                                    gw_v[:, f // 2, f % 2],
                                    start=(kt == 0 and f == 0),
                                    stop=(kt == KK - 1 and f == 3))

                    oc = wpool.tile([64, CH], f32, tag="oc")
                    nc.scalar.activation(oc[:], dcn_ps[:], Act.Identity,
                                         bias=b3_sb[:, 0:1], scale=1.0)
                    ob = wpool.tile([64, CH], f32, tag="ob")
                    lrelu_stt(ob[:], oc[:])
                    nc.sync.dma_start(out_d[:, c * CH:(c + 1) * CH], ob[:])

    nc.compile()
    return nc


def _prep_inputs(nbr, ref, w_off1, b_off1, w_om, b_om, w_dcn, b_dcn):
    """Build the 8 per-core input dicts."""
    bf = ml_dtypes.bfloat16
    in_maps = []
    # weights shared by all cores
    w1 = np.zeros((128, KK * 64), np.float32)
    for kt in range(KK):
        ky, kx = kt // 3, kt % 3
        w1[:, kt * 64:(kt + 1) * 64] = w_off1[:, :, ky, kx].T
    w2 = np.zeros((64, 3 * KK * 72), np.float32)
    for f in range(3):
        for kt in range(KK):
            ky, kx = kt // 3, kt % 3
            blk = np.zeros((64, 72), np.float32)
            for k in range(KK):
                for g in range(G):
                    blk[:, k * 8 + g] = w_om[f * 72 + g * KK + k, :, ky, kx]
            w2[:, (f * KK + kt) * 72:(f * KK + kt + 1) * 72] = blk
    w3 = np.zeros((128, KK * 64), np.float32)
    wd = w_dcn.reshape(64, G, CG, 3, 3)
    for kt in range(KK):
        ky, kx = kt // 3, kt % 3
        blk = np.zeros((128, 64), np.float32)
        for g in range(G):
            for j in range(CG):
                blk[16 * g + j, :] = wd[:, g, j, ky, kx]
        w3[:, kt * 64:(kt + 1) * 64] = blk

    wrep = np.zeros((72, KK * 128), np.float32)
    for kt in range(KK):
        for m in range(128):
            wrep[kt * 8 + m // 16, kt * 128 + m] = 1.0

    dy = np.repeat(np.arange(3) - 1, 3).astype(np.float32)
    dx = np.tile(np.arange(3) - 1, 3).astype(np.float32)
    by = np.zeros((72, 1), np.float32)
    bx = np.zeros((72, 1), np.float32)
    bm = np.zeros((72, 1), np.float32)
    for k in range(KK):
        for g in range(G):
            p = k * 8 + g
            by[p, 0] = b_om[0 * 72 + g * KK + k] + dy[k] + GP
            bx[p, 0] = b_om[1 * 72 + g * KK + k] + dx[k] + GP
            bm[p, 0] = b_om[2 * 72 + g * KK + k]
    b1 = b_off1.reshape(64, 1).astype(np.float32)
    b3 = b_dcn.reshape(64, 1).astype(np.float32)

    # ramp for chunk 0: y_local*NPR + x/2 over positions q = r*128 + x
    q = np.arange(CH, dtype=np.float32)
    rampc = (q // W) * NPR + (q % W) * 0.5
    rampc = np.broadcast_to(rampc, (128, CH)).astype(np.float32).copy()

    for core in range(N_CORES):
        b = core // 2
        s = (core % 2) * HALF
        # conv1 input: concat channels, rows [s-2, s+66), zero pad, 130 cols
        ci = np.zeros((128, 68, 130), np.float32)
        cat = np.concatenate([nbr[b], ref[b]], axis=0)
        r_lo, r_hi = s - 2, s + 66
        src_lo, src_hi = max(r_lo, 0), min(r_hi, H)
        ci[:, src_lo - r_lo: src_hi - r_lo, 1:129] = cat[:, src_lo:src_hi, :]

        # gather source: guard-padded per-channel image, row-interleaved A|B pairs
        pad = np.zeros((128, HP, WP), np.float32)
        g_lo, g_hi = s - GP, s + HALF + GP
        sg_lo, sg_hi = max(g_lo, 0), min(g_hi, H)
        for g in range(G):
            for j in range(16):
                ch = CG * g + (j % CG)
                pad[16 * g + j, sg_lo - g_lo: sg_hi - g_lo, GP:GP + W] = \
                    nbr[b, ch, sg_lo:sg_hi, :]
        # A pairs: (2t, 2t+1); B pairs: (2t+1, 2t+2)
        shift = np.concatenate([pad[:, :, 1:], np.zeros((128, HP, 1), np.float32)],
                               axis=2)
        ab = np.concatenate([pad.reshape(128, HP, NPR // 2, 2),
                             shift.reshape(128, HP, NPR // 2, 2)], axis=2)
        nbrab = ab.reshape(128, NE_AB * 2)

        e0 = np.full((64, 1), 0.0 if s == 0 else 1.0, np.float32)
        e65 = np.full((64, 1), 0.0 if s + HALF == H else 1.0, np.float32)
        in_maps.append(dict(
            conv_in=ci.reshape(128, -1).astype(bf),
            nbrab=nbrab.astype(bf),
            w1=w1.astype(bf), w2=w2.astype(bf), w3=w3.astype(bf), wrep=wrep,
            by=by, bx=bx, bm=bm, b1=b1, b3=b3, e0=e0, e65=e65,
            rampc=rampc, c156=np.full((72, 1), float(NPR), np.float32),
        ))
    return in_maps


def kernel(**inputs):
    global _compiled
    from concourse.bass_utils import run_bass_kernel_spmd

    if _compiled is None:
        _compiled = _build_program()
    nc = _compiled

    in_maps = _prep_inputs(
        inputs["nbr_fea_l"], inputs["ref_fea_l"], inputs["w_off1"],
        inputs["b_off1"], inputs["w_om"], inputs["b_om"],
        inputs["w_dcn"], inputs["b_dcn"])

    res = run_bass_kernel_spmd(nc, in_maps, core_ids=list(range(N_CORES)))
    out = np.zeros((B, NF, H, W), np.float32)
    for core in range(N_CORES):
        b = core // 2
        s = (core % 2) * HALF
        out[b, :, s:s + HALF, :] = res.results[core]["out"].reshape(64, HALF, W)
    return out


if __name__ == "__main__":
    print("smoke build only")
    _build_program()
    print("build ok")
